# revision 30
# baseline (speedup 1.0000x reference)
"""Multi-head causal attention (RoPE) Trainium2 kernel, SPMD over 8 NeuronCores.

Sharding: core c handles batch b = c // 4 and head-group g = c % 4
(4 heads of 128 dims each => 512 output features per core). Fully
embarrassingly parallel - no collectives.

Per-core device kernel (all matmuls bf16, f32 PSUM accumulation):
  phase A: Q^T, K^T ([d, s] layout) and V ([s, d] layout) projections from
           x^T (host-pretransposed logits). q/k bias is added during the
           PSUM->SBUF evacuation (ScalarE activation per-partition bias);
           v bias via a K=1 matmul row. RoPE applied to q/k in half-rotated
           layout (host permutes W rows so rotation pairs are (i, i+64) =>
           partition-half swap via SBUF-SBUF DMA).
  phase B: per head, scores^T[sk, sq] = (K^T tile)^T @ Q^T chunk, exp on
           ScalarE (scale folded in), causal handled by tile skipping +
           right-narrowed diagonal tiles + one 0/1 triangular mask multiply,
           PV matmul with a ones-column appended to V so the softmax
           denominator accumulates for free in PSUM col 128, final
           normalize on VectorE, DMA out.
"""

import sys

import numpy as np
import ml_dtypes

for _p in ("/opt/trn_rl_repo",):
    if _p not in sys.path:
        sys.path.insert(0, _p)

B, S, E = 2, 2048, 2048
H, D = 16, 128
P = 128
HPC = 4            # heads per core
F = HPC * D        # 512 projection features per core
NCORES = 8
NE = E // P        # 16 contraction tiles
NSQ = S // P       # 16 query row-tiles
NCH = S // 512     # 4 query chunks of 512
ROPE_BASE = 10000.0
SM_SCALE = 1.0 / float(np.sqrt(D))
BF16 = ml_dtypes.bfloat16

_compiled = None
LAST_RESULT = None

# interleaved (0,1),(2,3).. pairs -> half layout (i, i+64): new_i = old 2i,
# new_{i+64} = old 2i+1. Applied to q/k weight rows per head; scores are
# invariant since the same permutation hits q and k.
_PERM = np.concatenate([np.arange(0, D, 2), np.arange(1, D, 2)])


def _rope_tables():
    inv = ROPE_BASE ** (-np.arange(0, D, 2, dtype=np.float64) / D)      # [64]
    ang = np.arange(S, dtype=np.float64)[None, :] * inv[:, None]        # [64, S]
    cos, sin = np.cos(ang), np.sin(ang)
    cosf = np.concatenate([cos, cos], axis=0).astype(BF16)              # [128, S]
    ssin = np.concatenate([-sin, sin], axis=0).astype(BF16)
    return cosf, ssin


def _mask_tile():
    # mask[p, f] = 1 iff f >= p (lower-triangle-inclusive block)
    f = np.arange(512)[None, :]
    p = np.arange(P)[:, None]
    return (f >= p).astype(np.float32).astype(BF16)


def _build():
    import concourse.mybir as mybir
    import concourse.tile as tile
    from concourse import bacc

    fdt = mybir.dt.float32
    bdt = mybir.dt.bfloat16
    Exp = mybir.ActivationFunctionType.Exp
    Ident = mybir.ActivationFunctionType.Identity

    nc = bacc.Bacc("TRN2", target_bir_lowering=False, debug=False,
                   num_devices=NCORES)

    xt = nc.dram_tensor("xt", [E, S], bdt, kind="ExternalInput").ap()
    wqt = nc.dram_tensor("wqt", [E, F], bdt, kind="ExternalInput").ap()
    wkt = nc.dram_tensor("wkt", [E, F], bdt, kind="ExternalInput").ap()
    wvt = nc.dram_tensor("wvt", [E, F], bdt, kind="ExternalInput").ap()
    bqd = nc.dram_tensor("bqd", [P, HPC], fdt, kind="ExternalInput").ap()
    bkd = nc.dram_tensor("bkd", [P, HPC], fdt, kind="ExternalInput").ap()
    bvd = nc.dram_tensor("bvd", [1, F], bdt, kind="ExternalInput").ap()
    cosd = nc.dram_tensor("cosd", [P, S], bdt, kind="ExternalInput").ap()
    ssind = nc.dram_tensor("ssind", [P, S], bdt, kind="ExternalInput").ap()
    maskd = nc.dram_tensor("maskd", [P, 512], bdt, kind="ExternalInput").ap()
    outd = nc.dram_tensor("out", [S, F], fdt, kind="ExternalOutput").ap()

    with tile.TileContext(nc) as tc:
        with (
            tc.tile_pool(name="const", bufs=1) as constp,
            tc.tile_pool(name="persist", bufs=1) as persist,
            tc.tile_pool(name="psum", bufs=2, space="PSUM") as psp,
        ):
            # small/late-needed constants go on the scalar (ACT) HWDGE queue
            # so they don't delay the x^T / weight stream on the sync queue.
            bqpt = constp.tile([P, HPC], fdt, tag="bqpt", name="bqpt")
            nc.scalar.dma_start(bqpt[:], bqd[:])
            bkpt = constp.tile([P, HPC], fdt, tag="bkpt", name="bkpt")
            nc.scalar.dma_start(bkpt[:], bkd[:])
            cos_sb = constp.tile([P, S], bdt, tag="cos", name="cos_sb")
            nc.scalar.dma_start(cos_sb[:], cosd[:])
            ssin_sb = constp.tile([P, S], bdt, tag="ssin", name="ssin_sb")
            nc.scalar.dma_start(ssin_sb[:], ssind[:])
            mask_sb = constp.tile([P, 512], bdt, tag="mask", name="mask_sb")
            nc.scalar.dma_start(mask_sb[:], maskd[:])
            bv_sb = constp.tile([1, F], bdt, tag="bv", name="bv_sb")
            nc.scalar.dma_start(bv_sb[:], bvd[:])
            ones_row = constp.tile([1, P], bdt, tag="ones", name="ones_row")
            nc.vector.memset(ones_row[:], 1.0)

            qT = [persist.tile([P, S], bdt, tag=f"qT{h}", name=f"qT{h}")
                  for h in range(HPC)]
            kT = [persist.tile([P, S], bdt, tag=f"kT{h}", name=f"kT{h}")
                  for h in range(HPC)]
            vA = [[persist.tile([P, D + 1], bdt, tag=f"vA{h}_{j}",
                                name=f"vA{h}_{j}")
                   for j in range(NSQ)] for h in range(HPC)]

            # ---------------- phase A: projections + RoPE ----------------
            with (
                tc.tile_pool(name="xtp", bufs=1) as xp,
                tc.tile_pool(name="wp", bufs=2) as wp,
                tc.tile_pool(name="evac", bufs=3) as ep,
            ):
                # interleave W_q and x^T tiles on the sync queue so the PE
                # can start consuming (wq[e], xt[e]) pairs as they land.
                xts = [None] * NE
                wqs = [None] * NE
                for e in range(NE):
                    wtile = wp.tile([P, F], bdt, tag=f"w{e}", name=f"q{e}")
                    nc.sync.dma_start(wtile[:], wqt[P * e:P * (e + 1), :])
                    wqs[e] = wtile
                    t = xp.tile([P, S], bdt, tag=f"x{e}", name=f"x{e}")
                    nc.sync.dma_start(t[:], xt[P * e:P * (e + 1), :])
                    xts[e] = t

                def load_w(wd, pfx):
                    wts = []
                    for e in range(NE):
                        wtile = wp.tile([P, F], bdt, tag=f"w{e}",
                                        name=f"{pfx}{e}")
                        nc.sync.dma_start(wtile[:], wd[P * e:P * (e + 1), :])
                        wts.append(wtile)
                    return wts

                def qk_proj(wts, bias_pt, dst, pfx):
                    for h in range(HPC):
                        pss = [psp.tile([P, 512], fdt, tag=f"p{c}",
                                        name=f"{pfx}ps{h}_{c}")
                               for c in range(NCH)]
                        for e in range(NE):
                            for c in range(NCH):
                                nc.tensor.matmul(
                                    pss[c][:],
                                    wts[e][:, P * h:P * (h + 1)],
                                    xts[e][:, 512 * c:512 * (c + 1)],
                                    start=(e == 0), stop=(e == NE - 1))
                        for c in range(NCH):
                            cs = slice(512 * c, 512 * (c + 1))
                            xs = ep.tile([P, 512], bdt, tag="xs",
                                         name=f"{pfx}xs{h}_{c}")
                            nc.scalar.activation(xs[:], pss[c][:], Ident,
                                                 bias=bias_pt[:, h:h + 1])
                            sw = ep.tile([P, 512], bdt, tag="sw",
                                         name=f"{pfx}sw{h}_{c}")
                            nc.scalar.dma_start(sw[0:64, :], xs[64:128, :])
                            nc.scalar.dma_start(sw[64:128, :], xs[0:64, :])
                            t1 = ep.tile([P, 512], bdt, tag="t1",
                                         name=f"{pfx}t1_{h}_{c}")
                            nc.vector.tensor_mul(t1[:], xs[:], cos_sb[:, cs])
                            t2 = ep.tile([P, 512], bdt, tag="t2",
                                         name=f"{pfx}t2_{h}_{c}")
                            nc.vector.tensor_mul(t2[:], sw[:], ssin_sb[:, cs])
                            nc.vector.tensor_add(dst[h][:, cs], t1[:], t2[:])

                qk_proj(wqs, bqpt, qT, "q")
                qk_proj(load_w(wkt, "k"), bkpt, kT, "k")

                wts = load_w(wvt, "v")
                for j in range(NSQ):
                    ps = psp.tile([P, 512], fdt, tag=f"p{j % NCH}",
                                  name=f"vps{j}")
                    nc.tensor.matmul(ps[:], ones_row[:], bv_sb[:],
                                     start=True, stop=False)
                    for e in range(NE):
                        nc.tensor.matmul(
                            ps[:], xts[e][:, P * j:P * (j + 1)], wts[e][:],
                            start=False, stop=(e == NE - 1))
                    for h in range(HPC):
                        nc.vector.tensor_copy(vA[h][j][:, 0:D],
                                              ps[:, D * h:D * (h + 1)])
                        nc.vector.memset(vA[h][j][:, D:D + 1], 1.0)

            # ---------------- phase B: causal attention ----------------
            with (
                tc.tile_pool(name="etp", bufs=3) as etp,
                tc.tile_pool(name="ost", bufs=4) as osp,
            ):
                for h in range(HPC):
                    for c in range(NCH):
                        nt = 4 * c + 4
                        # diagonal tiles first: their exp+mask chain is the
                        # longest, so start it before the bulk tiles.
                        order = list(range(4 * c, nt)) + list(range(0, 4 * c))
                        sel = [None] * nt
                        off = [0] * nt
                        for t in order:
                            diag = t >= 4 * c
                            o = P * (t % 4) if diag else 0
                            w = 512 - o
                            off[t] = o
                            cs = slice(512 * c + o, 512 * (c + 1))
                            ps_sc = psp.tile([P, w], fdt, tag=f"p{t % 2}",
                                             name=f"sc{h}_{c}_{t}")
                            nc.tensor.matmul(
                                ps_sc[:], kT[h][:, P * t:P * (t + 1)],
                                qT[h][:, cs], start=True, stop=True)
                            et = etp.tile([P, w], bdt, tag=f"et{t}",
                                          name=f"et{h}_{c}_{t}")
                            nc.scalar.activation(et[:], ps_sc[:], Exp,
                                                 scale=SM_SCALE)
                            if diag:
                                etm = etp.tile([P, w], bdt,
                                               tag=f"etm{t % 4}",
                                               name=f"etm{h}_{c}_{t}")
                                nc.vector.tensor_mul(etm[:], et[:],
                                                     mask_sb[:, 0:w])
                                sel[t] = etm
                            else:
                                sel[t] = et
                        for jj in range(4):
                            j = 4 * c + jj
                            po = psp.tile([P, D + 1], fdt,
                                          tag=f"p{2 + jj % 2}",
                                          name=f"po{h}_{j}")
                            for t in range(j + 1):
                                lo = P * jj - off[t]
                                nc.tensor.matmul(
                                    po[:], sel[t][:, lo:lo + P],
                                    vA[h][t][:],
                                    start=(t == 0), stop=(t == j))
                            rec = osp.tile([P, 1], fdt, tag="rec",
                                           name=f"rec{h}_{j}")
                            nc.vector.reciprocal(rec[:], po[:, D:D + 1])
                            ot = osp.tile([P, D], fdt, tag="ot",
                                          name=f"ot{h}_{j}")
                            nc.vector.tensor_scalar_mul(ot[:], po[:, 0:D],
                                                        rec[:])
                            nc.sync.dma_start(
                                outd[P * j:P * (j + 1), D * h:D * (h + 1)],
                                ot[:])

    nc.compile()
    return nc


def get_compiled():
    global _compiled
    if _compiled is None:
        _compiled = _build()
    return _compiled


def make_in_maps(logits, Wq, bq, Wk, bk, Wv, bv):
    cosf, ssin = _rope_tables()
    maskm = _mask_tile()
    xts = [np.ascontiguousarray(np.asarray(logits)[b].T).astype(BF16)
           for b in range(B)]

    def permW(Wm, rows):
        Wp = np.asarray(Wm)[rows].reshape(HPC, D, E)[:, _PERM, :].reshape(F, E)
        return np.ascontiguousarray(Wp.T).astype(BF16)

    def permb(bvec, rows):
        # [128, HPC] f32: column h = permuted bias of head h
        return np.ascontiguousarray(
            np.asarray(bvec)[rows].reshape(HPC, D)[:, _PERM].T
        ).astype(np.float32)

    in_maps = []
    for core in range(NCORES):
        b, g = divmod(core, 4)
        rows = slice(F * g, F * (g + 1))
        in_maps.append({
            "xt": xts[b],
            "wqt": permW(Wq, rows),
            "wkt": permW(Wk, rows),
            "wvt": np.ascontiguousarray(np.asarray(Wv)[rows].T).astype(BF16),
            "bqd": permb(bq, rows),
            "bkd": permb(bk, rows),
            "bvd": np.asarray(bv)[rows].reshape(1, F).astype(BF16),
            "cosd": cosf,
            "ssind": ssin,
            "maskd": maskm,
        })
    return in_maps


def kernel(logits, Wq, bq, Wk, bk, Wv, bv, **_ignored):
    global LAST_RESULT
    from concourse.bass_utils import run_bass_kernel_spmd

    nc = get_compiled()
    in_maps = make_in_maps(logits, Wq, bq, Wk, bk, Wv, bv)
    res = run_bass_kernel_spmd(nc, in_maps, list(range(NCORES)))
    LAST_RESULT = res
    out = np.empty((B, S, H * D), dtype=np.float32)
    for core in range(NCORES):
        b, g = divmod(core, 4)
        out[b, :, F * g:F * (g + 1)] = res.results[core]["out"]
    return out
